# revision 28
# baseline (speedup 1.0000x reference)
"""ACR-GNN forward on 8 Trainium2 NeuronCores.

Strategy:
  - Nodes sharded contiguously: core c owns nodes [c*6250, (c+1)*6250).
  - Edges bucketed by dst owner; sorted by dst window (128 dsts); split into
    two groups by src half (int16 gather index limit: 32768 rows/table).
  - Per layer: node features live in a replicated node-major bf16 DRAM table
    [50000,128] (AllGather each layer).  Messages h[src] fetched with
    gpsimd.dma_gather (256B rows).  Segment-sum over sorted dst via one-hot
    S-matrix matmuls on TensorE accumulating per-128-dst-window in PSUM
    (aggr comes out feat-major, f32).  S built on DVE: is_equal(iota, doff).
  - Combine: out[fo, nodes] = VwT.T@h + AwT.T@aggr + RGT.T@B_T, relu+bias on
    ScalarE eviction.  BN stats via DVE reduce + 1KB AllReduce; normalize on
    DVE.  Readout graph-sums via bf16 B one-hot matmuls on node-major
    transposed tiles (also written back as the next table shard).
"""

import numpy as np

N = 50000
E = 800000
G = 64
IN_DIM = 64
HID = 128
OUT_DIM = 2
L = 3
BN_EPS = 1e-5

NCORES = 8
NLOC = N // NCORES            # 6250
WIN = 128
NWIN = (NLOC + WIN - 1) // WIN  # 49
NT = NWIN                     # node-major 128-chunks per core
T0 = 32768                    # rows in gather sub-table 0
T1 = N - T0                   # 17232
SENT = 16384.0                # dst-offset sentinel for padded edges
GC = 1024                     # edges per dma_gather call (single_packet limit)
SB = 8                        # S-matrix chunks built per DVE op
CWIN = 512                    # combine window (moving free dim)
NCW = (NLOC + CWIN - 1) // CWIN  # 13


def _bf16(a):
    import ml_dtypes
    return np.asarray(a, dtype=np.float32).astype(ml_dtypes.bfloat16)


# ----------------------------------------------------------------- host prep
def _prep(x, edge_index, batch, V_w, V_b, A_w, A_b, R_w, R_b,
          bn_gamma, bn_beta, lin_w, lin_b):
    src = np.asarray(edge_index[0], dtype=np.int64)
    dst = np.asarray(edge_index[1], dtype=np.int64)
    batch = np.asarray(batch, dtype=np.int64)
    x = np.asarray(x, dtype=np.float32)

    owner = dst // NLOC
    dstl = dst - owner * NLOC
    grp = (src >= T0).astype(np.int64)
    win = dstl // WIN
    key = (owner * 2 + grp) * NWIN + win
    order = np.argsort(key, kind="stable")
    cnt = np.bincount(key, minlength=NCORES * 2 * NWIN).reshape(NCORES, 2, NWIN)
    # shared (SPMD) chunk counts per (group, window): cross-core max
    K = ((cnt + 127) // 128).max(axis=0)          # [2, NWIN]
    K = np.maximum(K, 1)                          # keep >=1 chunk per window
    NCH = K.sum(axis=1)                           # chunks per group
    SG = NCH * 128                                # padded edges per group
    base_ck = np.zeros((2, NWIN), dtype=np.int64)
    base_ck[:, 1:] = np.cumsum(K, axis=1)[:, :-1]

    src_sorted = src[order]
    dstl_sorted = dstl[order]
    grp_off = np.zeros(NCORES * 2 * NWIN + 1, dtype=np.int64)
    grp_off[1:] = np.cumsum(np.bincount(key, minlength=NCORES * 2 * NWIN))

    # full padded node table for layer 0
    h0tab = np.zeros((N, HID), dtype=np.float32)
    h0tab[:, :IN_DIM] = x
    h0tab_bf = _bf16(h0tab)

    in_maps = []
    for c in range(NCORES):
        m = {}
        m["h0tab"] = h0tab_bf
        xl = h0tab[c * NLOC:(c + 1) * NLOC]               # [6250,128]
        m["xfm"] = np.ascontiguousarray(xl.T)             # [128,6250] f32
        xnm = np.zeros((128, NT * HID), dtype=np.float32)  # node-major chunks
        for t in range(NT):
            n0, n1 = t * 128, min((t + 1) * 128, NLOC)
            xnm[: n1 - n0, t * HID:(t + 1) * HID] = xl[n0:n1]
        m["xnm"] = _bf16(xnm)

        for g in range(2):
            idx = np.zeros(SG[g], dtype=np.int16)
            dof = np.full(SG[g], SENT, dtype=np.float32)
            for w in range(NWIN):
                a, b = grp_off[(c * 2 + g) * NWIN + w], grp_off[(c * 2 + g) * NWIN + w + 1]
                p0 = base_ck[g, w] * 128
                ln = b - a
                idx[p0:p0 + ln] = (src_sorted[a:b] - g * T0).astype(np.int16)
                dof[p0:p0 + ln] = (dstl_sorted[a:b] - w * WIN).astype(np.float32)
            # wrap idx into [16, SG/16] then replicate to 128 partitions
            iw = idx.reshape(SG[g] // 16, 16).T
            m[f"gidx{g}"] = np.ascontiguousarray(np.tile(iw, (8, 1)))
            m[f"doff{g}"] = np.ascontiguousarray(dof.reshape(NCH[g], 128).T)

        bl = batch[c * NLOC:(c + 1) * NLOC]
        bnm = np.zeros((128, NT * G), dtype=np.float32)
        for t in range(NT):
            n0, n1 = t * 128, min((t + 1) * 128, NLOC)
            loc = np.arange(n1 - n0)
            blk = np.zeros((128, G), dtype=np.float32)
            blk[loc, bl[n0:n1]] = 1.0
            bnm[:, t * G:(t + 1) * G] = blk
        m["bnm"] = _bf16(bnm)
        bt = np.zeros((128, NLOC), dtype=np.float32)
        bt[bl, np.arange(NLOC)] = 1.0
        deg = np.bincount(dstl[owner == c], minlength=NLOC).astype(np.float32)
        bt[G, :] = deg
        m["bt"] = np.ascontiguousarray(bt)

        wts = np.zeros((HID, 9 * HID), dtype=np.float32)
        for l in range(L):
            wts[:, (l * 3 + 0) * HID:(l * 3 + 1) * HID] = V_w[l].T
            wts[:, (l * 3 + 1) * HID:(l * 3 + 2) * HID] = A_w[l].T
            wts[:, (l * 3 + 2) * HID:(l * 3 + 3) * HID] = R_w[l].T
        m["wts"] = wts
        aux = np.zeros((HID, 9), dtype=np.float32)
        for l in range(L):
            aux[:, l] = V_b[l] + A_b[l] + R_b[l]
            aux[:, 3 + l] = bn_gamma[l]
            aux[:, 6 + l] = bn_beta[l]
        m["aux"] = aux
        gcnt = np.bincount(batch, minlength=G).astype(np.float32)
        m["gcnt"] = np.ascontiguousarray(np.tile(gcnt[None, :], (128, 1)))
        m["lint"] = np.ascontiguousarray(lin_w.T.astype(np.float32))  # [128,2]
        m["linb"] = np.ascontiguousarray(
            np.asarray(lin_b, dtype=np.float32).reshape(OUT_DIM, 1))
        in_maps.append(m)

    meta = dict(K=K, NCH=NCH, SG=SG, base_ck=base_ck)
    return in_maps, meta


# -------------------------------------------------------------- bass builder
def _build(meta):
    import os
    import concourse.bass as bass
    import concourse.bacc as bacc
    import concourse.mybir as mybir
    import concourse.tile as tile

    SKIP_GATHER = os.environ.get("GNN_SKIP_GATHER", "") == "1"
    NL = int(os.environ.get("GNN_NL", str(L)))
    PHASE = int(os.environ.get("GNN_PHASE", "4"))

    K = meta["K"]; NCH = meta["NCH"]; SG = meta["SG"]; base_ck = meta["base_ck"]
    f32 = mybir.dt.float32
    bf16 = mybir.dt.bfloat16
    i16 = mybir.dt.int16
    AT = mybir.ActivationFunctionType
    OP = mybir.AluOpType

    nc = bacc.Bacc("TRN2", target_bir_lowering=False, debug=False,
                   num_devices=NCORES)

    h0tab = nc.dram_tensor("h0tab", [N, HID], bf16, kind="ExternalInput")
    xfm_d = nc.dram_tensor("xfm", [HID, NLOC], f32, kind="ExternalInput")
    xnm_d = nc.dram_tensor("xnm", [128, NT * HID], bf16, kind="ExternalInput")
    gidx_d = [nc.dram_tensor(f"gidx{g}", [128, SG[g] // 16], i16,
                             kind="ExternalInput") for g in range(2)]
    doff_d = [nc.dram_tensor(f"doff{g}", [128, NCH[g]], f32,
                             kind="ExternalInput") for g in range(2)]
    bnm_d = nc.dram_tensor("bnm", [128, NT * G], bf16, kind="ExternalInput")
    bt_d = nc.dram_tensor("bt", [128, NLOC], f32, kind="ExternalInput")
    wts_d = nc.dram_tensor("wts", [HID, 9 * HID], f32, kind="ExternalInput")
    aux_d = nc.dram_tensor("aux", [HID, 9], f32, kind="ExternalInput")
    gcnt_d = nc.dram_tensor("gcnt", [128, G], f32, kind="ExternalInput")
    lint_d = nc.dram_tensor("lint", [HID, OUT_DIM], f32, kind="ExternalInput")
    linb_d = nc.dram_tensor("linb", [OUT_DIM, 1], f32, kind="ExternalInput")
    out_d = nc.dram_tensor("out", [OUT_DIM, NLOC], f32, kind="ExternalOutput")

    rg_all = [list(range(NCORES))]

    with tile.TileContext(nc) as tc:
        with (
            tc.tile_pool(name="const", bufs=1) as cpool,
            tc.tile_pool(name="big", bufs=1) as bpool,
            tc.tile_pool(name="msg", bufs=4) as mpool,
            tc.tile_pool(name="sweep", bufs=2) as spool,
            tc.tile_pool(name="trp", bufs=3) as tpool,
            tc.tile_pool(name="fold", bufs=2) as fpool,
            tc.tile_pool(name="outw", bufs=2) as opool,
            tc.tile_pool(name="psA", bufs=2, space="PSUM") as psA,
            tc.tile_pool(name="psC", bufs=2, space="PSUM") as psC,
            tc.tile_pool(name="psT", bufs=2, space="PSUM") as psT,
            tc.tile_pool(name="psS", bufs=2, space="PSUM") as psS,
            tc.tile_pool(name="dram", bufs=1, space="DRAM") as dpool,
            tc.tile_pool(name="dramT", bufs=2, space="DRAM") as dTpool,
        ):
            # ---------------- constants / weights to SBUF
            wts = cpool.tile([HID, 9 * HID], f32)
            nc.sync.dma_start(wts[:], wts_d[:])
            aux = cpool.tile([HID, 9], f32)
            nc.sync.dma_start(aux[:], aux_d[:])
            lint = cpool.tile([HID, OUT_DIM], f32)
            nc.sync.dma_start(lint[:], lint_d[:])
            linb = cpool.tile([OUT_DIM, 1], f32)
            nc.sync.dma_start(linb[:], linb_d[:])
            bnm = cpool.tile([128, NT * G], bf16)
            nc.sync.dma_start(bnm[:], bnm_d[:])
            bt = cpool.tile([128, NLOC], f32)
            nc.sync.dma_start(bt[:], bt_d[:])
            gidx = [cpool.tile([128, SG[g] // 16], i16, name=f"gidx{g}s")
                    for g in range(2)]
            doff = [cpool.tile([128, NCH[g]], f32, name=f"doff{g}s")
                    for g in range(2)]
            for g in range(2):
                nc.sync.dma_start(gidx[g][:], gidx_d[g][:])
                nc.sync.dma_start(doff[g][:], doff_d[g][:])

            iota = cpool.tile([128, 128], f32)
            nc.gpsimd.iota(iota[:], pattern=[[1, 128]], base=0,
                           channel_multiplier=0,
                           allow_small_or_imprecise_dtypes=True)
            iotac = cpool.tile([128, 1], f32)
            nc.gpsimd.iota(iotac[:], pattern=[[1, 1]], base=0,
                           channel_multiplier=1,
                           allow_small_or_imprecise_dtypes=True)
            ident = cpool.tile([128, 128], f32)
            nc.vector.tensor_scalar(ident[:], iota[:], iotac[:], None,
                                    OP.is_equal)

            gcnt = cpool.tile([128, G], f32)
            nc.sync.dma_start(gcnt[:], gcnt_d[:])

            # ---------------- persistent activations
            h_a = bpool.tile([HID, NLOC], f32)        # ping-pong h (feat-major)
            nc.sync.dma_start(h_a[:], xfm_d[:])
            h_b = bpool.tile([HID, NLOC], f32)
            aggr = bpool.tile([HID, NLOC], f32)
            rgt = bpool.tile([128, HID], f32)         # (R_w[l] @ G).T, rows G.. zero
            stats = bpool.tile([HID, 2], f32)
            statsr = bpool.tile([HID, 2], f32)
            sq_acc = bpool.tile([HID, NCW], f32)
            sq_scr = bpool.tile([HID, CWIN], f32)
            sfac = bpool.tile([HID, 8], f32)          # bn scalars scratch
            gsb = bpool.tile([G, HID], f32)
            grr = bpool.tile([G, HID], f32)
            gfm = bpool.tile([HID, G], f32)
            gfm2 = bpool.tile([HID, G], f32)
            gtmp = bpool.tile([HID, G], f32)
            rgs = bpool.tile([HID, G], f32)
            pac = bpool.tile([HID, 1], f32)

            # DRAM bounce buffers (collective outs: one writer each)
            g_ins = [dpool.tile([G, HID], f32, name=f"g_in{l}")
                     for l in range(L)]
            g_outs = [dpool.tile([G, HID], f32, addr_space="Shared",
                                 name=f"g_out{l}") for l in range(L)]
            st_ins = [dpool.tile([HID, 2], f32, name=f"st_in{l}")
                      for l in range(L)]
            st_outs = [dpool.tile([HID, 2], f32, addr_space="Shared",
                                  name=f"st_out{l}") for l in range(L)]

            def rg_start(l):
                nc.sync.dma_start(g_ins[l][:], gsb[:])
                nc.gpsimd.collective_compute(
                    "AllReduce", mybir.AluOpType.add, replica_groups=rg_all,
                    ins=[g_ins[l].opt()], outs=[g_outs[l].opt()])
                nc.sync.dma_start(grr[:], g_outs[l][:])

            def rg_finish(l):
                """Apply pending BN affine (sfac); rgt[0:G] <- (R_l@G_true).T
                G node-major [G,HID] -> feat-major [HID,G]"""
                tp = psT.tile([128, G], f32, name="tpg", tag="trp")
                nc.tensor.transpose(tp[:, :G], grr[:], ident[:G, :G])
                nc.vector.tensor_copy(gfm[:], tp[:, :G])
                # G_true = scale*G_raw + shift*graph_count
                nc.vector.tensor_scalar(gfm2[:], gfm[:], sfac[:, 6:7], None,
                                        OP.mult)
                nc.vector.tensor_scalar(gtmp[:], gcnt[:], sfac[:, 7:8], None,
                                        OP.mult)
                nc.vector.tensor_add(gfm2[:], gfm2[:], gtmp[:])
                rgp = psT.tile([HID, G], f32, name="rgp", tag="trp")
                nc.tensor.matmul(rgp[:], wts[:, (l * 3 + 2) * HID:(l * 3 + 3) * HID],
                                 gfm2[:], start=True, stop=True)
                nc.vector.tensor_copy(rgs[:], rgp[:])
                tp2 = psT.tile([G, HID], f32, name="tpg2", tag="trp")
                nc.tensor.transpose(tp2[:G, :], rgs[:], ident[:])
                nc.vector.tensor_copy(rgt[0:G, :], tp2[:G, :])

            def fold_weights(l):
                """Fold pending BN affine into layer-l V/A weights + bias;
                rgt[G] row <- (A_l @ shift) (pairs with bt deg row)."""
                wf = fpool.tile([HID, 2 * HID], f32, name="wf", tag="wf")
                nc.vector.tensor_scalar(
                    wf[:, 0:HID], wts[:, (l * 3 + 0) * HID:(l * 3 + 1) * HID],
                    sfac[:, 6:7], None, OP.mult)
                nc.vector.tensor_scalar(
                    wf[:, HID:2 * HID],
                    wts[:, (l * 3 + 1) * HID:(l * 3 + 2) * HID],
                    sfac[:, 6:7], None, OP.mult)
                pb = psT.tile([HID, 1], f32, name="pb", tag="trp")
                nc.tensor.matmul(pb[:], wts[:, (l * 3 + 0) * HID:(l * 3 + 1) * HID],
                                 sfac[:, 7:8], start=True, stop=True)
                bias_f = fpool.tile([HID, 1], f32, name="biasf", tag="biasf")
                nc.vector.tensor_add(bias_f[:], aux[:, l:l + 1], pb[:])
                pa = psT.tile([HID, 1], f32, name="pa", tag="trp")
                nc.tensor.matmul(pa[:], wts[:, (l * 3 + 1) * HID:(l * 3 + 2) * HID],
                                 sfac[:, 7:8], start=True, stop=True)
                nc.vector.tensor_copy(pac[:], pa[:])
                prow = psT.tile([1, HID], f32, name="prow", tag="trp")
                nc.tensor.transpose(prow[:1, :], pac[:], ident[:])
                nc.vector.tensor_copy(rgt[G:G + 1, :], prow[:1, :])
                return wf, bias_f

            nc.vector.memset(rgt[:], 0.0)
            nc.vector.memset(sfac[:, 6:7], 1.0)
            nc.vector.memset(sfac[:, 7:8], 0.0)
            # prologue: readout partials of layer-0 input (node-major tiles)
            gps = psS.tile([G, HID], f32, name="gps", tag="gps")
            for t in range(NT):
                xt = tpool.tile([128, HID], bf16, name="xt", tag="trt")
                nc.sync.dma_start(xt[:], xnm_d[:, t * HID:(t + 1) * HID])
                nc.tensor.matmul(gps[:], bnm[:, t * G:(t + 1) * G], xt[:],
                                 start=(t == 0), stop=(t == NT - 1))
            nc.vector.tensor_copy(gsb[:], gps[:])
            rg_start(0)
            rg_finish(0)
            wf, bias_f = fold_weights(0)

            h_cur, h_nxt = h_a, h_b
            tabs = []
            for l in range(NL):
                # ---------------- gather + segment sum (aggr)
                if l == 0:
                    tab0, tab1 = h0tab[0:T0, :], h0tab[T0:N, :]
                else:
                    tab0, tab1 = tabs[-1][0:T0, :], tabs[-1][T0:N, :]

                ngath = [(SG[g] + GC - 1) // GC for g in range(2)]
                msgs = [[None] * ngath[g] for g in range(2)]
                stil = [[None] * ((NCH[g] + SB - 1) // SB) for g in range(2)]

                def get_msg(g, gi, tab0=tab0, tab1=tab1, msgs=msgs):
                    if msgs[g][gi] is None:
                        nidx = min(GC, SG[g] - gi * GC)
                        mt = mpool.tile([128, GC // 128, HID], bf16,
                                        name=f"msg{g}", tag=f"msg{g}")
                        tabg = tab0 if g == 0 else tab1
                        nc.gpsimd.dma_gather(
                            mt[:, :nidx // 128, :], tabg,
                            gidx[g][:, gi * (GC // 16):gi * (GC // 16) + nidx // 16],
                            nidx, nidx, HID, single_packet=True)
                        msgs[g][gi] = mt
                    return msgs[g][gi]

                def get_s(g, si, stil=stil):
                    if stil[g][si] is None:
                        nck = min(SB, NCH[g] - si * SB)
                        st = spool.tile([128, SB, 128], bf16,
                                        name=f"stl{g}", tag=f"stl{g}")
                        dslice = doff[g][:, si * SB:si * SB + nck]
                        din = dslice.unsqueeze(2).broadcast_to([128, nck, 128])
                        iin = iota[:].unsqueeze(1).broadcast_to([128, nck, 128])
                        nc.vector.tensor_tensor(st[:, :nck, :], din, iin,
                                                OP.is_equal)
                        stil[g][si] = st
                    return stil[g][si]

                if SKIP_GATHER:
                    nc.vector.memset(aggr[:], 0.0)
                else:
                    for w in range(NWIN):
                        ps = psA.tile([HID, WIN], f32, name="aggwin",
                                      tag="aggwin")
                        tot = int(K[0][w] + K[1][w])
                        done = 0
                        for g in range(2):
                            for k in range(int(K[g][w])):
                                ck = int(base_ck[g][w]) + k
                                mt = get_msg(g, ck // (GC // 128))
                                st = get_s(g, ck // SB)
                                nc.tensor.matmul(
                                    ps[:],
                                    mt[:, ck % (GC // 128), :],
                                    st[:, ck % SB, :],
                                    start=(done == 0), stop=(done == tot - 1))
                                done += 1
                        n0 = w * WIN
                        n1 = min(NLOC, n0 + WIN)
                        nc.vector.tensor_copy(aggr[:, n0:n1], ps[:, :n1 - n0])

                # ---------------- combine + relu (h_nxt = relu(...), un-BN'd)
                for wi in range(NCW):
                    n0 = wi * CWIN
                    n1 = min(NLOC, n0 + CWIN)
                    nn = n1 - n0
                    pc = psC.tile([HID, CWIN], f32, name="cmb", tag="cmb")
                    nc.tensor.matmul(pc[:, :nn], wf[:, 0:HID],
                                     h_cur[:, n0:n1], start=True, stop=False)
                    nc.tensor.matmul(pc[:, :nn], wf[:, HID:2 * HID],
                                     aggr[:, n0:n1], start=False, stop=False)
                    nc.tensor.matmul(pc[:, :nn], rgt[:],
                                     bt[:, n0:n1], start=False, stop=True)
                    nc.scalar.activation(h_nxt[:, n0:n1], pc[:, :nn],
                                         AT.Relu, bias=bias_f[:],
                                         scale=1.0)
                    nc.vector.tensor_tensor(
                        sq_scr[:, :nn], h_nxt[:, n0:n1],
                        h_nxt[:, n0:n1], OP.mult)
                    nc.vector.tensor_reduce(
                        sq_acc[:, wi:wi + 1], sq_scr[:, :nn],
                        mybir.AxisListType.X, OP.add)

                if PHASE >= 3:
                    # --------- node-major tiles: G-partials (+ table shard)
                    if l < L - 1:
                        shard = dTpool.tile([NLOC, HID], bf16, name="shard",
                                            tag="shard")
                    gps = psS.tile([G, HID], f32, name="gps", tag="gps")
                    for t in range(NT):
                        n0 = t * 128
                        n1 = min(NLOC, n0 + 128)
                        nn = n1 - n0
                        tp = psT.tile([128, 128], f32, name="trp", tag="trp")
                        nc.tensor.transpose(tp[:nn, :], h_nxt[:, n0:n1],
                                            ident[:])
                        tr = tpool.tile([128, HID], bf16, name="trs",
                                        tag="trt")
                        nc.vector.tensor_copy(tr[:nn, :], tp[:nn, :])
                        nc.tensor.matmul(gps[:], bnm[:, t * G:(t + 1) * G],
                                         tr[:], start=(t == 0),
                                         stop=(t == NT - 1))
                        if l < L - 1:
                            nc.sync.dma_start(shard[n0:n1, :], tr[:nn, :])
                    nc.vector.tensor_copy(gsb[:], gps[:])
                    if PHASE >= 4 and l < L - 1:
                        rg_start(l + 1)
                    if PHASE >= 4 and l < L - 1:
                        tabn = dTpool.tile([N, HID], bf16, name="tabn",
                                           tag="tabn", addr_space="Shared")
                        nc.gpsimd.collective_compute(
                            "AllGather", mybir.AluOpType.bypass,
                            replica_groups=rg_all,
                            ins=[shard.opt()], outs=[tabn.opt()])
                        tabs.append(tabn)

                if PHASE < 2:
                    h_cur, h_nxt = h_nxt, h_cur
                    continue
                # ---------------- BN stats -> scale/shift (pending affine)
                nc.vector.tensor_reduce(stats[:, 0:1], h_nxt[:],
                                        mybir.AxisListType.X, OP.add)
                nc.vector.tensor_reduce(stats[:, 1:2], sq_acc[:],
                                        mybir.AxisListType.X, OP.add)
                nc.sync.dma_start(st_ins[l][:], stats[:])
                nc.gpsimd.collective_compute(
                    "AllReduce", mybir.AluOpType.add, replica_groups=rg_all,
                    ins=[st_ins[l].opt()], outs=[st_outs[l].opt()])
                nc.sync.dma_start(statsr[:], st_outs[l][:])
                # mean, E[x2], var, std, rstd, scale, shift
                nc.vector.tensor_scalar(sfac[:, 0:2], statsr[:], 1.0 / N, None,
                                        OP.mult)
                nc.vector.tensor_tensor(
                    sfac[:, 2:3], sfac[:, 0:1], sfac[:, 0:1], OP.mult)
                nc.vector.tensor_scalar(sfac[:, 3:4], sfac[:, 1:2],
                                        sfac[:, 2:3], BN_EPS, OP.subtract,
                                        OP.add)
                nc.scalar.sqrt(sfac[:, 4:5], sfac[:, 3:4])
                nc.vector.reciprocal(sfac[:, 5:6], sfac[:, 4:5])
                nc.vector.tensor_scalar(sfac[:, 6:7], aux[:, 3 + l:4 + l],
                                        sfac[:, 5:6], None, OP.mult)
                # shift = beta - mean*scale
                nc.vector.tensor_scalar(sfac[:, 7:8], sfac[:, 0:1],
                                        sfac[:, 6:7], None, OP.mult)
                nc.vector.tensor_sub(sfac[:, 7:8], aux[:, 6 + l:7 + l],
                                     sfac[:, 7:8])

                if PHASE >= 4 and l < L - 1:
                    rg_finish(l + 1)
                if l < L - 1:
                    wf, bias_f = fold_weights(l + 1)
                h_cur, h_nxt = h_nxt, h_cur

            # ---------------- final linear (BN affine folded in)
            lf = fpool.tile([HID, OUT_DIM], f32, name="lf", tag="biasf")
            nc.vector.tensor_scalar(lf[:], lint[:], sfac[:, 6:7], None,
                                    OP.mult)
            plb = psT.tile([OUT_DIM, 1], f32, name="plb", tag="trp")
            nc.tensor.matmul(plb[:], lint[:], sfac[:, 7:8], start=True,
                             stop=True)
            lbf = fpool.tile([OUT_DIM, 1], f32, name="lbf", tag="pac2")
            nc.vector.tensor_add(lbf[:], linb[:], plb[:])
            for wi in range(NCW):
                n0 = wi * CWIN
                n1 = min(NLOC, n0 + CWIN)
                nn = n1 - n0
                po = psC.tile([OUT_DIM, CWIN], f32, name="outp", tag="cmb")
                nc.tensor.matmul(po[:, :nn], lf[:], h_cur[:, n0:n1],
                                 start=True, stop=True)
                ow = opool.tile([OUT_DIM, CWIN], f32, name="ow", tag="ow")
                nc.scalar.activation(ow[:, :nn], po[:, :nn], AT.Identity,
                                     bias=lbf[:], scale=1.0)
                nc.sync.dma_start(out_d[:, n0:n1], ow[:, :nn])

    nc.compile()
    return nc


# ------------------------------------------------------------------- driver
def kernel(**inputs):
    from concourse.bass_utils import run_bass_kernel_spmd

    in_maps, meta = _prep(**inputs)
    nc = _build(meta)
    res = run_bass_kernel_spmd(nc, in_maps, core_ids=list(range(NCORES)))
    out = np.zeros((N, OUT_DIM), dtype=np.float32)
    for c in range(NCORES):
        out[c * NLOC:(c + 1) * NLOC, :] = res.results[c]["out"].T
    return out


# revision 29
# speedup vs baseline: 1.2298x; 1.2298x over previous
"""ACR-GNN forward on 8 Trainium2 NeuronCores.

Strategy:
  - Nodes sharded contiguously: core c owns nodes [c*6250, (c+1)*6250).
  - Edges bucketed by dst owner; sorted by dst window (128 dsts); split into
    two groups by src half (int16 gather index limit: 32768 rows/table).
  - Per layer: node features live in a replicated node-major bf16 DRAM table
    [50000,128] (AllGather each layer).  Messages h[src] fetched with
    gpsimd.dma_gather (256B rows).  Segment-sum over sorted dst via one-hot
    S-matrix matmuls on TensorE accumulating per-128-dst-window in PSUM
    (aggr comes out feat-major, f32).  S built on DVE: is_equal(iota, doff).
  - Combine: out[fo, nodes] = VwT.T@h + AwT.T@aggr + RGT.T@B_T, relu+bias on
    ScalarE eviction.  BN stats via DVE reduce + 1KB AllReduce; normalize on
    DVE.  Readout graph-sums via bf16 B one-hot matmuls on node-major
    transposed tiles (also written back as the next table shard).
"""

import numpy as np

N = 50000
E = 800000
G = 64
IN_DIM = 64
HID = 128
OUT_DIM = 2
L = 3
BN_EPS = 1e-5

NCORES = 8
NLOC = N // NCORES            # 6250
WIN = 128
NWIN = (NLOC + WIN - 1) // WIN  # 49
NT = NWIN                     # node-major 128-chunks per core
T0 = 32768                    # rows in gather sub-table 0
T1 = N - T0                   # 17232
SENT = 16384.0                # dst-offset sentinel for padded edges
GC = 1024                     # edges per dma_gather call (single_packet limit)
SB = 8                        # S-matrix chunks built per DVE op
CWIN = 512                    # combine window (moving free dim)
NCW = (NLOC + CWIN - 1) // CWIN  # 13


def _bf16(a):
    import ml_dtypes
    return np.asarray(a, dtype=np.float32).astype(ml_dtypes.bfloat16)


# ----------------------------------------------------------------- host prep
def _prep(x, edge_index, batch, V_w, V_b, A_w, A_b, R_w, R_b,
          bn_gamma, bn_beta, lin_w, lin_b):
    src = np.asarray(edge_index[0], dtype=np.int64)
    dst = np.asarray(edge_index[1], dtype=np.int64)
    batch = np.asarray(batch, dtype=np.int64)
    x = np.asarray(x, dtype=np.float32)

    owner = dst // NLOC
    dstl = dst - owner * NLOC
    grp = (src >= T0).astype(np.int64)
    win = dstl // WIN
    key = (owner * 2 + grp) * NWIN + win
    order = np.argsort(key, kind="stable")
    cnt = np.bincount(key, minlength=NCORES * 2 * NWIN).reshape(NCORES, 2, NWIN)
    # shared (SPMD) chunk counts per (group, window): cross-core max
    K = ((cnt + 127) // 128).max(axis=0)          # [2, NWIN]
    K = np.maximum(K, 1)                          # keep >=1 chunk per window
    NCH = K.sum(axis=1)                           # chunks per group
    SG = NCH * 128                                # padded edges per group
    base_ck = np.zeros((2, NWIN), dtype=np.int64)
    base_ck[:, 1:] = np.cumsum(K, axis=1)[:, :-1]

    src_sorted = src[order]
    dstl_sorted = dstl[order]
    grp_off = np.zeros(NCORES * 2 * NWIN + 1, dtype=np.int64)
    grp_off[1:] = np.cumsum(np.bincount(key, minlength=NCORES * 2 * NWIN))

    # full padded node table for layer 0
    h0tab = np.zeros((N, HID), dtype=np.float32)
    h0tab[:, :IN_DIM] = x
    h0tab_bf = _bf16(h0tab)

    in_maps = []
    for c in range(NCORES):
        m = {}
        m["h0tab"] = h0tab_bf
        xl = h0tab[c * NLOC:(c + 1) * NLOC]               # [6250,128]
        m["xfm"] = np.ascontiguousarray(xl.T)             # [128,6250] f32
        xnm = np.zeros((128, NT * HID), dtype=np.float32)  # node-major chunks
        for t in range(NT):
            n0, n1 = t * 128, min((t + 1) * 128, NLOC)
            xnm[: n1 - n0, t * HID:(t + 1) * HID] = xl[n0:n1]
        m["xnm"] = _bf16(xnm)

        for g in range(2):
            idx = np.zeros(SG[g], dtype=np.int16)
            dof = np.full(SG[g], SENT, dtype=np.float32)
            for w in range(NWIN):
                a, b = grp_off[(c * 2 + g) * NWIN + w], grp_off[(c * 2 + g) * NWIN + w + 1]
                p0 = base_ck[g, w] * 128
                ln = b - a
                idx[p0:p0 + ln] = (src_sorted[a:b] - g * T0).astype(np.int16)
                dof[p0:p0 + ln] = (dstl_sorted[a:b] - w * WIN).astype(np.float32)
            # wrap idx into [16, SG/16] then replicate to 128 partitions
            iw = idx.reshape(SG[g] // 16, 16).T
            m[f"gidx{g}"] = np.ascontiguousarray(np.tile(iw, (8, 1)))
            m[f"doff{g}"] = _bf16(np.ascontiguousarray(dof.reshape(NCH[g], 128).T))

        bl = batch[c * NLOC:(c + 1) * NLOC]
        bnm = np.zeros((128, NT * G), dtype=np.float32)
        for t in range(NT):
            n0, n1 = t * 128, min((t + 1) * 128, NLOC)
            loc = np.arange(n1 - n0)
            blk = np.zeros((128, G), dtype=np.float32)
            blk[loc, bl[n0:n1]] = 1.0
            bnm[:, t * G:(t + 1) * G] = blk
        m["bnm"] = _bf16(bnm)
        bt = np.zeros((128, NLOC), dtype=np.float32)
        bt[bl, np.arange(NLOC)] = 1.0
        deg = np.bincount(dstl[owner == c], minlength=NLOC).astype(np.float32)
        bt[G, :] = deg
        m["bt"] = np.ascontiguousarray(bt)

        wts = np.zeros((HID, 9 * HID), dtype=np.float32)
        for l in range(L):
            wts[:, (l * 3 + 0) * HID:(l * 3 + 1) * HID] = V_w[l].T
            wts[:, (l * 3 + 1) * HID:(l * 3 + 2) * HID] = A_w[l].T
            wts[:, (l * 3 + 2) * HID:(l * 3 + 3) * HID] = R_w[l].T
        m["wts"] = wts
        aux = np.zeros((HID, 9), dtype=np.float32)
        for l in range(L):
            aux[:, l] = V_b[l] + A_b[l] + R_b[l]
            aux[:, 3 + l] = bn_gamma[l]
            aux[:, 6 + l] = bn_beta[l]
        m["aux"] = aux
        gcnt = np.bincount(batch, minlength=G).astype(np.float32)
        m["gcnt"] = np.ascontiguousarray(np.tile(gcnt[None, :], (128, 1)))
        m["lint"] = np.ascontiguousarray(lin_w.T.astype(np.float32))  # [128,2]
        m["linb"] = np.ascontiguousarray(
            np.asarray(lin_b, dtype=np.float32).reshape(OUT_DIM, 1))
        in_maps.append(m)

    meta = dict(K=K, NCH=NCH, SG=SG, base_ck=base_ck)
    return in_maps, meta


# -------------------------------------------------------------- bass builder
def _build(meta):
    import os
    import concourse.bass as bass
    import concourse.bacc as bacc
    import concourse.mybir as mybir
    import concourse.tile as tile

    SKIP_GATHER = os.environ.get("GNN_SKIP_GATHER", "") == "1"
    NL = int(os.environ.get("GNN_NL", str(L)))
    PHASE = int(os.environ.get("GNN_PHASE", "4"))

    K = meta["K"]; NCH = meta["NCH"]; SG = meta["SG"]; base_ck = meta["base_ck"]
    f32 = mybir.dt.float32
    bf16 = mybir.dt.bfloat16
    i16 = mybir.dt.int16
    AT = mybir.ActivationFunctionType
    OP = mybir.AluOpType

    nc = bacc.Bacc("TRN2", target_bir_lowering=False, debug=False,
                   num_devices=NCORES)

    h0tab = nc.dram_tensor("h0tab", [N, HID], bf16, kind="ExternalInput")
    xfm_d = nc.dram_tensor("xfm", [HID, NLOC], f32, kind="ExternalInput")
    xnm_d = nc.dram_tensor("xnm", [128, NT * HID], bf16, kind="ExternalInput")
    gidx_d = [nc.dram_tensor(f"gidx{g}", [128, SG[g] // 16], i16,
                             kind="ExternalInput") for g in range(2)]
    doff_d = [nc.dram_tensor(f"doff{g}", [128, NCH[g]], bf16,
                             kind="ExternalInput") for g in range(2)]
    bnm_d = nc.dram_tensor("bnm", [128, NT * G], bf16, kind="ExternalInput")
    bt_d = nc.dram_tensor("bt", [128, NLOC], f32, kind="ExternalInput")
    wts_d = nc.dram_tensor("wts", [HID, 9 * HID], f32, kind="ExternalInput")
    aux_d = nc.dram_tensor("aux", [HID, 9], f32, kind="ExternalInput")
    gcnt_d = nc.dram_tensor("gcnt", [128, G], f32, kind="ExternalInput")
    lint_d = nc.dram_tensor("lint", [HID, OUT_DIM], f32, kind="ExternalInput")
    linb_d = nc.dram_tensor("linb", [OUT_DIM, 1], f32, kind="ExternalInput")
    out_d = nc.dram_tensor("out", [OUT_DIM, NLOC], f32, kind="ExternalOutput")

    rg_all = [list(range(NCORES))]

    with tile.TileContext(nc) as tc:
        with (
            tc.tile_pool(name="const", bufs=1) as cpool,
            tc.tile_pool(name="big", bufs=1) as bpool,
            tc.tile_pool(name="msg", bufs=4) as mpool,
            tc.tile_pool(name="sweep", bufs=2) as spool,
            tc.tile_pool(name="trp", bufs=3) as tpool,
            tc.tile_pool(name="fold", bufs=2) as fpool,
            tc.tile_pool(name="outw", bufs=2) as opool,
            tc.tile_pool(name="psA", bufs=2, space="PSUM") as psA,
            tc.tile_pool(name="psC", bufs=2, space="PSUM") as psC,
            tc.tile_pool(name="psT", bufs=2, space="PSUM") as psT,
            tc.tile_pool(name="psS", bufs=2, space="PSUM") as psS,
            tc.tile_pool(name="dram", bufs=1, space="DRAM") as dpool,
            tc.tile_pool(name="dramT", bufs=2, space="DRAM") as dTpool,
        ):
            # ---------------- constants / weights to SBUF
            wts = cpool.tile([HID, 9 * HID], f32)
            nc.sync.dma_start(wts[:], wts_d[:])
            aux = cpool.tile([HID, 9], f32)
            nc.sync.dma_start(aux[:], aux_d[:])
            lint = cpool.tile([HID, OUT_DIM], f32)
            nc.sync.dma_start(lint[:], lint_d[:])
            linb = cpool.tile([OUT_DIM, 1], f32)
            nc.sync.dma_start(linb[:], linb_d[:])
            bnm = cpool.tile([128, NT * G], bf16)
            nc.sync.dma_start(bnm[:], bnm_d[:])
            bt = cpool.tile([128, NLOC], f32)
            nc.sync.dma_start(bt[:], bt_d[:])
            gidx = [cpool.tile([128, SG[g] // 16], i16, name=f"gidx{g}s")
                    for g in range(2)]
            doff = [cpool.tile([128, NCH[g]], bf16, name=f"doff{g}s")
                    for g in range(2)]
            for g in range(2):
                nc.sync.dma_start(gidx[g][:], gidx_d[g][:])
                nc.sync.dma_start(doff[g][:], doff_d[g][:])

            iota = cpool.tile([128, 128], f32)
            nc.gpsimd.iota(iota[:], pattern=[[1, 128]], base=0,
                           channel_multiplier=0,
                           allow_small_or_imprecise_dtypes=True)
            iotac = cpool.tile([128, 1], f32)
            nc.gpsimd.iota(iotac[:], pattern=[[1, 1]], base=0,
                           channel_multiplier=1,
                           allow_small_or_imprecise_dtypes=True)
            ident = cpool.tile([128, 128], f32)
            nc.vector.tensor_scalar(ident[:], iota[:], iotac[:], None,
                                    OP.is_equal)
            iota_bf = cpool.tile([128, 128], bf16)
            nc.scalar.copy(iota_bf[:], iota[:])

            gcnt = cpool.tile([128, G], f32)
            nc.sync.dma_start(gcnt[:], gcnt_d[:])

            # ---------------- persistent activations
            h_a = bpool.tile([HID, NLOC], f32)        # ping-pong h (feat-major)
            nc.sync.dma_start(h_a[:], xfm_d[:])
            h_b = bpool.tile([HID, NLOC], f32)
            aggr = bpool.tile([HID, NLOC], f32)
            rgt = bpool.tile([128, HID], f32)         # (R_w[l] @ G).T, rows G.. zero
            stats = bpool.tile([HID, 2], f32)
            statsr = bpool.tile([HID, 2], f32)
            sq_acc = bpool.tile([HID, NCW], f32)
            sq_scr = bpool.tile([HID, CWIN], f32)
            sfac = bpool.tile([HID, 8], f32)          # bn scalars scratch
            gsb = bpool.tile([G, HID], f32)
            grr = bpool.tile([G, HID], f32)
            gfm = bpool.tile([HID, G], f32)
            gfm2 = bpool.tile([HID, G], f32)
            gtmp = bpool.tile([HID, G], f32)
            rgs = bpool.tile([HID, G], f32)
            pac = bpool.tile([HID, 1], f32)

            # DRAM bounce buffers (collective outs: one writer each)
            g_ins = [dpool.tile([G, HID], f32, name=f"g_in{l}")
                     for l in range(L)]
            g_outs = [dpool.tile([G, HID], f32, addr_space="Shared",
                                 name=f"g_out{l}") for l in range(L)]
            st_ins = [dpool.tile([HID, 2], f32, name=f"st_in{l}")
                      for l in range(L)]
            st_outs = [dpool.tile([HID, 2], f32, addr_space="Shared",
                                  name=f"st_out{l}") for l in range(L)]

            def rg_start(l):
                nc.sync.dma_start(g_ins[l][:], gsb[:])
                nc.gpsimd.collective_compute(
                    "AllReduce", mybir.AluOpType.add, replica_groups=rg_all,
                    ins=[g_ins[l].opt()], outs=[g_outs[l].opt()])
                nc.sync.dma_start(grr[:], g_outs[l][:])

            def rg_finish(l):
                """Apply pending BN affine (sfac); rgt[0:G] <- (R_l@G_true).T
                G node-major [G,HID] -> feat-major [HID,G]"""
                tp = psT.tile([128, G], f32, name="tpg", tag="trp")
                nc.tensor.transpose(tp[:, :G], grr[:], ident[:G, :G])
                nc.vector.tensor_copy(gfm[:], tp[:, :G])
                # G_true = scale*G_raw + shift*graph_count
                nc.vector.tensor_scalar(gfm2[:], gfm[:], sfac[:, 6:7], None,
                                        OP.mult)
                nc.vector.tensor_scalar(gtmp[:], gcnt[:], sfac[:, 7:8], None,
                                        OP.mult)
                nc.vector.tensor_add(gfm2[:], gfm2[:], gtmp[:])
                rgp = psT.tile([HID, G], f32, name="rgp", tag="trp")
                nc.tensor.matmul(rgp[:], wts[:, (l * 3 + 2) * HID:(l * 3 + 3) * HID],
                                 gfm2[:], start=True, stop=True)
                nc.vector.tensor_copy(rgs[:], rgp[:])
                tp2 = psT.tile([G, HID], f32, name="tpg2", tag="trp")
                nc.tensor.transpose(tp2[:G, :], rgs[:], ident[:])
                nc.vector.tensor_copy(rgt[0:G, :], tp2[:G, :])

            def fold_weights(l):
                """Fold pending BN affine into layer-l V/A weights + bias;
                rgt[G] row <- (A_l @ shift) (pairs with bt deg row)."""
                wf = fpool.tile([HID, 2 * HID], f32, name="wf", tag="wf")
                nc.vector.tensor_scalar(
                    wf[:, 0:HID], wts[:, (l * 3 + 0) * HID:(l * 3 + 1) * HID],
                    sfac[:, 6:7], None, OP.mult)
                nc.vector.tensor_scalar(
                    wf[:, HID:2 * HID],
                    wts[:, (l * 3 + 1) * HID:(l * 3 + 2) * HID],
                    sfac[:, 6:7], None, OP.mult)
                pb = psT.tile([HID, 1], f32, name="pb", tag="trp")
                nc.tensor.matmul(pb[:], wts[:, (l * 3 + 0) * HID:(l * 3 + 1) * HID],
                                 sfac[:, 7:8], start=True, stop=True)
                bias_f = fpool.tile([HID, 1], f32, name="biasf", tag="biasf")
                nc.vector.tensor_add(bias_f[:], aux[:, l:l + 1], pb[:])
                pa = psT.tile([HID, 1], f32, name="pa", tag="trp")
                nc.tensor.matmul(pa[:], wts[:, (l * 3 + 1) * HID:(l * 3 + 2) * HID],
                                 sfac[:, 7:8], start=True, stop=True)
                nc.vector.tensor_copy(pac[:], pa[:])
                prow = psT.tile([1, HID], f32, name="prow", tag="trp")
                nc.tensor.transpose(prow[:1, :], pac[:], ident[:])
                nc.vector.tensor_copy(rgt[G:G + 1, :], prow[:1, :])
                return wf, bias_f

            nc.vector.memset(rgt[:], 0.0)
            nc.vector.memset(sfac[:, 6:7], 1.0)
            nc.vector.memset(sfac[:, 7:8], 0.0)
            # prologue: readout partials of layer-0 input (node-major tiles)
            gps = psS.tile([G, HID], f32, name="gps", tag="gps")
            for t in range(NT):
                xt = tpool.tile([128, HID], bf16, name="xt", tag="trt")
                nc.sync.dma_start(xt[:], xnm_d[:, t * HID:(t + 1) * HID])
                nc.tensor.matmul(gps[:], bnm[:, t * G:(t + 1) * G], xt[:],
                                 start=(t == 0), stop=(t == NT - 1))
            nc.vector.tensor_copy(gsb[:], gps[:])
            rg_start(0)
            rg_finish(0)
            wf, bias_f = fold_weights(0)

            h_cur, h_nxt = h_a, h_b
            tabs = []
            for l in range(NL):
                # ---------------- gather + segment sum (aggr)
                if l == 0:
                    tab0, tab1 = h0tab[0:T0, :], h0tab[T0:N, :]
                else:
                    tab0, tab1 = tabs[-1][0:T0, :], tabs[-1][T0:N, :]

                ngath = [(SG[g] + GC - 1) // GC for g in range(2)]
                msgs = [[None] * ngath[g] for g in range(2)]
                stil = [[None] * ((NCH[g] + SB - 1) // SB) for g in range(2)]

                def get_msg(g, gi, tab0=tab0, tab1=tab1, msgs=msgs):
                    if msgs[g][gi] is None:
                        nidx = min(GC, SG[g] - gi * GC)
                        mt = mpool.tile([128, GC // 128, HID], bf16,
                                        name=f"msg{g}", tag=f"msg{g}")
                        tabg = tab0 if g == 0 else tab1
                        nc.gpsimd.dma_gather(
                            mt[:, :nidx // 128, :], tabg,
                            gidx[g][:, gi * (GC // 16):gi * (GC // 16) + nidx // 16],
                            nidx, nidx, HID, single_packet=True)
                        msgs[g][gi] = mt
                    return msgs[g][gi]

                def get_s(g, si, stil=stil):
                    if stil[g][si] is None:
                        nck = min(SB, NCH[g] - si * SB)
                        st = spool.tile([128, SB, 128], bf16,
                                        name=f"stl{g}", tag=f"stl{g}")
                        dslice = doff[g][:, si * SB:si * SB + nck]
                        din = dslice.unsqueeze(2).broadcast_to([128, nck, 128])
                        iin = iota_bf[:].unsqueeze(1).broadcast_to([128, nck, 128])
                        nc.vector.tensor_tensor(st[:, :nck, :], din, iin,
                                                OP.is_equal)
                        stil[g][si] = st
                    return stil[g][si]

                if SKIP_GATHER:
                    nc.vector.memset(aggr[:], 0.0)
                else:
                    for w in range(NWIN):
                        ps = psA.tile([HID, WIN], f32, name="aggwin",
                                      tag="aggwin")
                        tot = int(K[0][w] + K[1][w])
                        done = 0
                        for g in range(2):
                            for k in range(int(K[g][w])):
                                ck = int(base_ck[g][w]) + k
                                mt = get_msg(g, ck // (GC // 128))
                                st = get_s(g, ck // SB)
                                nc.tensor.matmul(
                                    ps[:],
                                    mt[:, ck % (GC // 128), :],
                                    st[:, ck % SB, :],
                                    start=(done == 0), stop=(done == tot - 1))
                                done += 1
                        n0 = w * WIN
                        n1 = min(NLOC, n0 + WIN)
                        nc.scalar.copy(aggr[:, n0:n1], ps[:, :n1 - n0])

                # ---------------- combine + relu (h_nxt = relu(...), un-BN'd)
                for wi in range(NCW):
                    n0 = wi * CWIN
                    n1 = min(NLOC, n0 + CWIN)
                    nn = n1 - n0
                    pc = psC.tile([HID, CWIN], f32, name="cmb", tag="cmb")
                    nc.tensor.matmul(pc[:, :nn], wf[:, 0:HID],
                                     h_cur[:, n0:n1], start=True, stop=False)
                    nc.tensor.matmul(pc[:, :nn], wf[:, HID:2 * HID],
                                     aggr[:, n0:n1], start=False, stop=False)
                    nc.tensor.matmul(pc[:, :nn], rgt[:],
                                     bt[:, n0:n1], start=False, stop=True)
                    nc.scalar.activation(h_nxt[:, n0:n1], pc[:, :nn],
                                         AT.Relu, bias=bias_f[:],
                                         scale=1.0)
                    nc.scalar.square(sq_scr[:, :nn], h_nxt[:, n0:n1])
                    nc.vector.tensor_reduce(
                        sq_acc[:, wi:wi + 1], sq_scr[:, :nn],
                        mybir.AxisListType.X, OP.add)

                if PHASE >= 3:
                    # --------- node-major tiles: G-partials (+ table shard)
                    if l < L - 1:
                        shard = dTpool.tile([NLOC, HID], bf16, name="shard",
                                            tag="shard")
                    gps = psS.tile([G, HID], f32, name="gps", tag="gps")
                    for t in range(NT):
                        n0 = t * 128
                        n1 = min(NLOC, n0 + 128)
                        nn = n1 - n0
                        tp = psT.tile([128, 128], f32, name="trp", tag="trp")
                        nc.tensor.transpose(tp[:nn, :], h_nxt[:, n0:n1],
                                            ident[:])
                        tr = tpool.tile([128, HID], bf16, name="trs",
                                        tag="trt")
                        nc.scalar.copy(tr[:nn, :], tp[:nn, :])
                        nc.tensor.matmul(gps[:], bnm[:, t * G:(t + 1) * G],
                                         tr[:], start=(t == 0),
                                         stop=(t == NT - 1))
                        if l < L - 1:
                            nc.sync.dma_start(shard[n0:n1, :], tr[:nn, :])
                    nc.scalar.copy(gsb[:], gps[:])
                    if PHASE >= 4 and l < L - 1:
                        rg_start(l + 1)
                    if PHASE >= 4 and l < L - 1:
                        tabn = dTpool.tile([N, HID], bf16, name="tabn",
                                           tag="tabn", addr_space="Shared")
                        nc.gpsimd.collective_compute(
                            "AllGather", mybir.AluOpType.bypass,
                            replica_groups=rg_all,
                            ins=[shard.opt()], outs=[tabn.opt()])
                        tabs.append(tabn)

                if PHASE < 2:
                    h_cur, h_nxt = h_nxt, h_cur
                    continue
                # ---------------- BN stats -> scale/shift (pending affine)
                nc.vector.tensor_reduce(stats[:, 0:1], h_nxt[:],
                                        mybir.AxisListType.X, OP.add)
                nc.vector.tensor_reduce(stats[:, 1:2], sq_acc[:],
                                        mybir.AxisListType.X, OP.add)
                nc.sync.dma_start(st_ins[l][:], stats[:])
                nc.gpsimd.collective_compute(
                    "AllReduce", mybir.AluOpType.add, replica_groups=rg_all,
                    ins=[st_ins[l].opt()], outs=[st_outs[l].opt()])
                nc.sync.dma_start(statsr[:], st_outs[l][:])
                # mean, E[x2], var, std, rstd, scale, shift
                nc.vector.tensor_scalar(sfac[:, 0:2], statsr[:], 1.0 / N, None,
                                        OP.mult)
                nc.vector.tensor_tensor(
                    sfac[:, 2:3], sfac[:, 0:1], sfac[:, 0:1], OP.mult)
                nc.vector.tensor_scalar(sfac[:, 3:4], sfac[:, 1:2],
                                        sfac[:, 2:3], BN_EPS, OP.subtract,
                                        OP.add)
                nc.scalar.sqrt(sfac[:, 4:5], sfac[:, 3:4])
                nc.vector.reciprocal(sfac[:, 5:6], sfac[:, 4:5])
                nc.vector.tensor_scalar(sfac[:, 6:7], aux[:, 3 + l:4 + l],
                                        sfac[:, 5:6], None, OP.mult)
                # shift = beta - mean*scale
                nc.vector.tensor_scalar(sfac[:, 7:8], sfac[:, 0:1],
                                        sfac[:, 6:7], None, OP.mult)
                nc.vector.tensor_sub(sfac[:, 7:8], aux[:, 6 + l:7 + l],
                                     sfac[:, 7:8])

                if PHASE >= 4 and l < L - 1:
                    rg_finish(l + 1)
                if l < L - 1:
                    wf, bias_f = fold_weights(l + 1)
                h_cur, h_nxt = h_nxt, h_cur

            # ---------------- final linear (BN affine folded in)
            lf = fpool.tile([HID, OUT_DIM], f32, name="lf", tag="biasf")
            nc.vector.tensor_scalar(lf[:], lint[:], sfac[:, 6:7], None,
                                    OP.mult)
            plb = psT.tile([OUT_DIM, 1], f32, name="plb", tag="trp")
            nc.tensor.matmul(plb[:], lint[:], sfac[:, 7:8], start=True,
                             stop=True)
            lbf = fpool.tile([OUT_DIM, 1], f32, name="lbf", tag="pac2")
            nc.vector.tensor_add(lbf[:], linb[:], plb[:])
            for wi in range(NCW):
                n0 = wi * CWIN
                n1 = min(NLOC, n0 + CWIN)
                nn = n1 - n0
                po = psC.tile([OUT_DIM, CWIN], f32, name="outp", tag="cmb")
                nc.tensor.matmul(po[:, :nn], lf[:], h_cur[:, n0:n1],
                                 start=True, stop=True)
                ow = opool.tile([OUT_DIM, CWIN], f32, name="ow", tag="ow")
                nc.scalar.activation(ow[:, :nn], po[:, :nn], AT.Identity,
                                     bias=lbf[:], scale=1.0)
                nc.sync.dma_start(out_d[:, n0:n1], ow[:, :nn])

    nc.compile()
    return nc


# ------------------------------------------------------------------- driver
def kernel(**inputs):
    from concourse.bass_utils import run_bass_kernel_spmd

    in_maps, meta = _prep(**inputs)
    nc = _build(meta)
    res = run_bass_kernel_spmd(nc, in_maps, core_ids=list(range(NCORES)))
    out = np.zeros((N, OUT_DIM), dtype=np.float32)
    for c in range(NCORES):
        out[c * NLOC:(c + 1) * NLOC, :] = res.results[c]["out"].T
    return out


# revision 30
# speedup vs baseline: 1.2402x; 1.0084x over previous
"""ACR-GNN forward on 8 Trainium2 NeuronCores.

Strategy:
  - Nodes sharded contiguously: core c owns nodes [c*6250, (c+1)*6250).
  - Edges bucketed by dst owner; sorted by dst window (128 dsts); split into
    two groups by src half (int16 gather index limit: 32768 rows/table).
  - Per layer: node features live in a replicated node-major bf16 DRAM table
    [50000,128] (AllGather each layer).  Messages h[src] fetched with
    gpsimd.dma_gather (256B rows).  Segment-sum over sorted dst via one-hot
    S-matrix matmuls on TensorE accumulating per-128-dst-window in PSUM
    (aggr comes out feat-major, f32).  S built on DVE: is_equal(iota, doff).
  - Combine: out[fo, nodes] = VwT.T@h + AwT.T@aggr + RGT.T@B_T, relu+bias on
    ScalarE eviction.  BN stats via DVE reduce + 1KB AllReduce; normalize on
    DVE.  Readout graph-sums via bf16 B one-hot matmuls on node-major
    transposed tiles (also written back as the next table shard).
"""

import numpy as np

N = 50000
E = 800000
G = 64
IN_DIM = 64
HID = 128
OUT_DIM = 2
L = 3
BN_EPS = 1e-5

NCORES = 8
NLOC = N // NCORES            # 6250
WIN = 128
NWIN = (NLOC + WIN - 1) // WIN  # 49
NT = NWIN                     # node-major 128-chunks per core
T0 = 32768                    # rows in gather sub-table 0
T1 = N - T0                   # 17232
SENT = 16384.0                # dst-offset sentinel for padded edges
GC = 1024                     # edges per dma_gather call (single_packet limit)
SB = 8                        # S-matrix chunks built per DVE op
CWIN = 512                    # combine window (moving free dim)
NCW = (NLOC + CWIN - 1) // CWIN  # 13


def _bf16(a):
    import ml_dtypes
    return np.asarray(a, dtype=np.float32).astype(ml_dtypes.bfloat16)


# ----------------------------------------------------------------- host prep
def _prep(x, edge_index, batch, V_w, V_b, A_w, A_b, R_w, R_b,
          bn_gamma, bn_beta, lin_w, lin_b):
    src = np.asarray(edge_index[0], dtype=np.int64)
    dst = np.asarray(edge_index[1], dtype=np.int64)
    batch = np.asarray(batch, dtype=np.int64)
    x = np.asarray(x, dtype=np.float32)

    owner = dst // NLOC
    dstl = dst - owner * NLOC
    grp = (src >= T0).astype(np.int64)
    win = dstl // WIN
    key = (owner * 2 + grp) * NWIN + win
    order = np.argsort(key, kind="stable")
    cnt = np.bincount(key, minlength=NCORES * 2 * NWIN).reshape(NCORES, 2, NWIN)
    # shared (SPMD) chunk counts per (group, window): cross-core max
    K = ((cnt + 127) // 128).max(axis=0)          # [2, NWIN]
    K = np.maximum(K, 1)                          # keep >=1 chunk per window
    NCH = K.sum(axis=1)                           # chunks per group
    SG = NCH * 128                                # padded edges per group
    base_ck = np.zeros((2, NWIN), dtype=np.int64)
    base_ck[:, 1:] = np.cumsum(K, axis=1)[:, :-1]

    src_sorted = src[order]
    dstl_sorted = dstl[order]
    grp_off = np.zeros(NCORES * 2 * NWIN + 1, dtype=np.int64)
    grp_off[1:] = np.cumsum(np.bincount(key, minlength=NCORES * 2 * NWIN))

    # full padded node table for layer 0
    h0tab = np.zeros((N, HID), dtype=np.float32)
    h0tab[:, :IN_DIM] = x
    h0tab_bf = _bf16(h0tab)

    in_maps = []
    for c in range(NCORES):
        m = {}
        m["h0tab"] = h0tab_bf
        xl = h0tab[c * NLOC:(c + 1) * NLOC]               # [6250,128]
        m["xfm"] = np.ascontiguousarray(xl.T)             # [128,6250] f32
        xnm = np.zeros((128, NT * HID), dtype=np.float32)  # node-major chunks
        for t in range(NT):
            n0, n1 = t * 128, min((t + 1) * 128, NLOC)
            xnm[: n1 - n0, t * HID:(t + 1) * HID] = xl[n0:n1]
        m["xnm"] = _bf16(xnm)

        for g in range(2):
            idx = np.zeros(SG[g], dtype=np.int16)
            dof = np.full(SG[g], SENT, dtype=np.float32)
            for w in range(NWIN):
                a, b = grp_off[(c * 2 + g) * NWIN + w], grp_off[(c * 2 + g) * NWIN + w + 1]
                p0 = base_ck[g, w] * 128
                ln = b - a
                idx[p0:p0 + ln] = (src_sorted[a:b] - g * T0).astype(np.int16)
                dof[p0:p0 + ln] = (dstl_sorted[a:b] - w * WIN).astype(np.float32)
            # wrap idx into [16, SG/16] then replicate to 128 partitions
            iw = idx.reshape(SG[g] // 16, 16).T
            m[f"gidx{g}"] = np.ascontiguousarray(np.tile(iw, (8, 1)))
            m[f"doff{g}"] = _bf16(np.ascontiguousarray(dof.reshape(NCH[g], 128).T))

        bl = batch[c * NLOC:(c + 1) * NLOC]
        bnm = np.zeros((128, NT * G), dtype=np.float32)
        for t in range(NT):
            n0, n1 = t * 128, min((t + 1) * 128, NLOC)
            loc = np.arange(n1 - n0)
            blk = np.zeros((128, G), dtype=np.float32)
            blk[loc, bl[n0:n1]] = 1.0
            bnm[:, t * G:(t + 1) * G] = blk
        m["bnm"] = _bf16(bnm)
        bt = np.zeros((128, NLOC), dtype=np.float32)
        bt[bl, np.arange(NLOC)] = 1.0
        deg = np.bincount(dstl[owner == c], minlength=NLOC).astype(np.float32)
        bt[G, :] = deg
        m["bt"] = np.ascontiguousarray(bt)

        wts = np.zeros((HID, 9 * HID), dtype=np.float32)
        for l in range(L):
            wts[:, (l * 3 + 0) * HID:(l * 3 + 1) * HID] = V_w[l].T
            wts[:, (l * 3 + 1) * HID:(l * 3 + 2) * HID] = A_w[l].T
            wts[:, (l * 3 + 2) * HID:(l * 3 + 3) * HID] = R_w[l].T
        m["wts"] = wts
        aux = np.zeros((HID, 9), dtype=np.float32)
        for l in range(L):
            aux[:, l] = V_b[l] + A_b[l] + R_b[l]
            aux[:, 3 + l] = bn_gamma[l]
            aux[:, 6 + l] = bn_beta[l]
        m["aux"] = aux
        gcnt = np.bincount(batch, minlength=G).astype(np.float32)
        m["gcnt"] = np.ascontiguousarray(np.tile(gcnt[None, :], (128, 1)))
        m["lint"] = np.ascontiguousarray(lin_w.T.astype(np.float32))  # [128,2]
        m["linb"] = np.ascontiguousarray(
            np.asarray(lin_b, dtype=np.float32).reshape(OUT_DIM, 1))
        in_maps.append(m)

    meta = dict(K=K, NCH=NCH, SG=SG, base_ck=base_ck)
    return in_maps, meta


# -------------------------------------------------------------- bass builder
def _build(meta):
    import os
    import concourse.bass as bass
    import concourse.bacc as bacc
    import concourse.mybir as mybir
    import concourse.tile as tile

    SKIP_GATHER = os.environ.get("GNN_SKIP_GATHER", "") == "1"
    NL = int(os.environ.get("GNN_NL", str(L)))
    PHASE = int(os.environ.get("GNN_PHASE", "4"))

    K = meta["K"]; NCH = meta["NCH"]; SG = meta["SG"]; base_ck = meta["base_ck"]
    f32 = mybir.dt.float32
    bf16 = mybir.dt.bfloat16
    i16 = mybir.dt.int16
    AT = mybir.ActivationFunctionType
    OP = mybir.AluOpType

    nc = bacc.Bacc("TRN2", target_bir_lowering=False, debug=False,
                   num_devices=NCORES)

    h0tab = nc.dram_tensor("h0tab", [N, HID], bf16, kind="ExternalInput")
    xfm_d = nc.dram_tensor("xfm", [HID, NLOC], f32, kind="ExternalInput")
    xnm_d = nc.dram_tensor("xnm", [128, NT * HID], bf16, kind="ExternalInput")
    gidx_d = [nc.dram_tensor(f"gidx{g}", [128, SG[g] // 16], i16,
                             kind="ExternalInput") for g in range(2)]
    doff_d = [nc.dram_tensor(f"doff{g}", [128, NCH[g]], bf16,
                             kind="ExternalInput") for g in range(2)]
    bnm_d = nc.dram_tensor("bnm", [128, NT * G], bf16, kind="ExternalInput")
    bt_d = nc.dram_tensor("bt", [128, NLOC], f32, kind="ExternalInput")
    wts_d = nc.dram_tensor("wts", [HID, 9 * HID], f32, kind="ExternalInput")
    aux_d = nc.dram_tensor("aux", [HID, 9], f32, kind="ExternalInput")
    gcnt_d = nc.dram_tensor("gcnt", [128, G], f32, kind="ExternalInput")
    lint_d = nc.dram_tensor("lint", [HID, OUT_DIM], f32, kind="ExternalInput")
    linb_d = nc.dram_tensor("linb", [OUT_DIM, 1], f32, kind="ExternalInput")
    out_d = nc.dram_tensor("out", [OUT_DIM, NLOC], f32, kind="ExternalOutput")

    rg_all = [list(range(NCORES))]

    with tile.TileContext(nc) as tc:
        with (
            tc.tile_pool(name="const", bufs=1) as cpool,
            tc.tile_pool(name="big", bufs=1) as bpool,
            tc.tile_pool(name="msg", bufs=4) as mpool,
            tc.tile_pool(name="sweep", bufs=2) as spool,
            tc.tile_pool(name="trp", bufs=3) as tpool,
            tc.tile_pool(name="fold", bufs=2) as fpool,
            tc.tile_pool(name="outw", bufs=2) as opool,
            tc.tile_pool(name="psA", bufs=2, space="PSUM") as psA,
            tc.tile_pool(name="psC", bufs=2, space="PSUM") as psC,
            tc.tile_pool(name="psT", bufs=2, space="PSUM") as psT,
            tc.tile_pool(name="psS", bufs=2, space="PSUM") as psS,
            tc.tile_pool(name="dram", bufs=1, space="DRAM") as dpool,
            tc.tile_pool(name="dramT", bufs=2, space="DRAM") as dTpool,
        ):
            # ---------------- constants / weights to SBUF
            wts = cpool.tile([HID, 9 * HID], f32)
            nc.sync.dma_start(wts[:], wts_d[:])
            aux = cpool.tile([HID, 9], f32)
            nc.sync.dma_start(aux[:], aux_d[:])
            lint = cpool.tile([HID, OUT_DIM], f32)
            nc.sync.dma_start(lint[:], lint_d[:])
            linb = cpool.tile([OUT_DIM, 1], f32)
            nc.sync.dma_start(linb[:], linb_d[:])
            bnm = cpool.tile([128, NT * G], bf16)
            nc.sync.dma_start(bnm[:], bnm_d[:])
            bt = cpool.tile([128, NLOC], f32)
            nc.sync.dma_start(bt[:], bt_d[:])
            gidx = [cpool.tile([128, SG[g] // 16], i16, name=f"gidx{g}s")
                    for g in range(2)]
            doff = [cpool.tile([128, NCH[g]], bf16, name=f"doff{g}s")
                    for g in range(2)]
            for g in range(2):
                nc.sync.dma_start(gidx[g][:], gidx_d[g][:])
                nc.sync.dma_start(doff[g][:], doff_d[g][:])

            iota = cpool.tile([128, 128], f32)
            nc.gpsimd.iota(iota[:], pattern=[[1, 128]], base=0,
                           channel_multiplier=0,
                           allow_small_or_imprecise_dtypes=True)
            iotac = cpool.tile([128, 1], f32)
            nc.gpsimd.iota(iotac[:], pattern=[[1, 1]], base=0,
                           channel_multiplier=1,
                           allow_small_or_imprecise_dtypes=True)
            ident = cpool.tile([128, 128], f32)
            nc.vector.tensor_scalar(ident[:], iota[:], iotac[:], None,
                                    OP.is_equal)
            iota_bf = cpool.tile([128, 128], bf16)
            nc.scalar.copy(iota_bf[:], iota[:])

            gcnt = cpool.tile([128, G], f32)
            nc.sync.dma_start(gcnt[:], gcnt_d[:])

            # ---------------- persistent activations
            h_a = bpool.tile([HID, NLOC], f32)        # ping-pong h (feat-major)
            nc.sync.dma_start(h_a[:], xfm_d[:])
            h_b = bpool.tile([HID, NLOC], f32)
            aggr = bpool.tile([HID, NLOC], f32)
            rgt = bpool.tile([128, HID], f32)         # (R_w[l] @ G).T, rows G.. zero
            stats = bpool.tile([HID, 2], f32)
            statsr = bpool.tile([HID, 2], f32)
            sq_acc = bpool.tile([HID, NCW], f32)
            sm_acc = bpool.tile([HID, NCW], f32)
            sq_scr = bpool.tile([HID, CWIN], f32)
            sfac = bpool.tile([HID, 8], f32)          # bn scalars scratch
            gsb = bpool.tile([G, HID], f32)
            grr = bpool.tile([G, HID], f32)
            gfm = bpool.tile([HID, G], f32)
            gfm2 = bpool.tile([HID, G], f32)
            gtmp = bpool.tile([HID, G], f32)
            rgs = bpool.tile([HID, G], f32)
            pac = bpool.tile([HID, 1], f32)

            # DRAM bounce buffers (collective outs: one writer each)
            g_ins = [dpool.tile([G, HID], f32, name=f"g_in{l}")
                     for l in range(L)]
            g_outs = [dpool.tile([G, HID], f32, addr_space="Shared",
                                 name=f"g_out{l}") for l in range(L)]
            st_ins = [dpool.tile([HID, 2], f32, name=f"st_in{l}")
                      for l in range(L)]
            st_outs = [dpool.tile([HID, 2], f32, addr_space="Shared",
                                  name=f"st_out{l}") for l in range(L)]

            def rg_start(l):
                nc.sync.dma_start(g_ins[l][:], gsb[:])
                nc.gpsimd.collective_compute(
                    "AllReduce", mybir.AluOpType.add, replica_groups=rg_all,
                    ins=[g_ins[l].opt()], outs=[g_outs[l].opt()])
                nc.sync.dma_start(grr[:], g_outs[l][:])

            def rg_finish(l):
                """Apply pending BN affine (sfac); rgt[0:G] <- (R_l@G_true).T
                G node-major [G,HID] -> feat-major [HID,G]"""
                tp = psT.tile([128, G], f32, name="tpg", tag="trp")
                nc.tensor.transpose(tp[:, :G], grr[:], ident[:G, :G])
                nc.vector.tensor_copy(gfm[:], tp[:, :G])
                # G_true = scale*G_raw + shift*graph_count
                nc.vector.tensor_scalar(gfm2[:], gfm[:], sfac[:, 6:7], None,
                                        OP.mult)
                nc.vector.tensor_scalar(gtmp[:], gcnt[:], sfac[:, 7:8], None,
                                        OP.mult)
                nc.vector.tensor_add(gfm2[:], gfm2[:], gtmp[:])
                rgp = psT.tile([HID, G], f32, name="rgp", tag="trp")
                nc.tensor.matmul(rgp[:], wts[:, (l * 3 + 2) * HID:(l * 3 + 3) * HID],
                                 gfm2[:], start=True, stop=True)
                nc.vector.tensor_copy(rgs[:], rgp[:])
                tp2 = psT.tile([G, HID], f32, name="tpg2", tag="trp")
                nc.tensor.transpose(tp2[:G, :], rgs[:], ident[:])
                nc.vector.tensor_copy(rgt[0:G, :], tp2[:G, :])

            def fold_weights(l):
                """Fold pending BN affine into layer-l V/A weights + bias;
                rgt[G] row <- (A_l @ shift) (pairs with bt deg row)."""
                wf = fpool.tile([HID, 2 * HID], f32, name="wf", tag="wf")
                nc.vector.tensor_scalar(
                    wf[:, 0:HID], wts[:, (l * 3 + 0) * HID:(l * 3 + 1) * HID],
                    sfac[:, 6:7], None, OP.mult)
                nc.vector.tensor_scalar(
                    wf[:, HID:2 * HID],
                    wts[:, (l * 3 + 1) * HID:(l * 3 + 2) * HID],
                    sfac[:, 6:7], None, OP.mult)
                pb = psT.tile([HID, 1], f32, name="pb", tag="trp")
                nc.tensor.matmul(pb[:], wts[:, (l * 3 + 0) * HID:(l * 3 + 1) * HID],
                                 sfac[:, 7:8], start=True, stop=True)
                bias_f = fpool.tile([HID, 1], f32, name="biasf", tag="biasf")
                nc.vector.tensor_add(bias_f[:], aux[:, l:l + 1], pb[:])
                pa = psT.tile([HID, 1], f32, name="pa", tag="trp")
                nc.tensor.matmul(pa[:], wts[:, (l * 3 + 1) * HID:(l * 3 + 2) * HID],
                                 sfac[:, 7:8], start=True, stop=True)
                nc.vector.tensor_copy(pac[:], pa[:])
                prow = psT.tile([1, HID], f32, name="prow", tag="trp")
                nc.tensor.transpose(prow[:1, :], pac[:], ident[:])
                nc.vector.tensor_copy(rgt[G:G + 1, :], prow[:1, :])
                return wf, bias_f

            nc.vector.memset(rgt[:], 0.0)
            nc.vector.memset(sfac[:, 6:7], 1.0)
            nc.vector.memset(sfac[:, 7:8], 0.0)
            # prologue: readout partials of layer-0 input (node-major tiles)
            gps = psS.tile([G, HID], f32, name="gps", tag="gps")
            for t in range(NT):
                xt = tpool.tile([128, HID], bf16, name="xt", tag="trt")
                nc.sync.dma_start(xt[:], xnm_d[:, t * HID:(t + 1) * HID])
                nc.tensor.matmul(gps[:], bnm[:, t * G:(t + 1) * G], xt[:],
                                 start=(t == 0), stop=(t == NT - 1))
            nc.vector.tensor_copy(gsb[:], gps[:])
            rg_start(0)
            rg_finish(0)
            wf, bias_f = fold_weights(0)

            h_cur, h_nxt = h_a, h_b
            tabs = []
            for l in range(NL):
                # ---------------- gather + segment sum (aggr)
                if l == 0:
                    tab0, tab1 = h0tab[0:T0, :], h0tab[T0:N, :]
                else:
                    tab0, tab1 = tabs[-1][0:T0, :], tabs[-1][T0:N, :]

                ngath = [(SG[g] + GC - 1) // GC for g in range(2)]
                msgs = [[None] * ngath[g] for g in range(2)]
                stil = [[None] * ((NCH[g] + SB - 1) // SB) for g in range(2)]

                def get_msg(g, gi, tab0=tab0, tab1=tab1, msgs=msgs):
                    if msgs[g][gi] is None:
                        nidx = min(GC, SG[g] - gi * GC)
                        mt = mpool.tile([128, GC // 128, HID], bf16,
                                        name=f"msg{g}", tag=f"msg{g}")
                        tabg = tab0 if g == 0 else tab1
                        nc.gpsimd.dma_gather(
                            mt[:, :nidx // 128, :], tabg,
                            gidx[g][:, gi * (GC // 16):gi * (GC // 16) + nidx // 16],
                            nidx, nidx, HID, single_packet=True)
                        msgs[g][gi] = mt
                    return msgs[g][gi]

                def get_s(g, si, stil=stil):
                    if stil[g][si] is None:
                        nck = min(SB, NCH[g] - si * SB)
                        st = spool.tile([128, SB, 128], bf16,
                                        name=f"stl{g}", tag=f"stl{g}")
                        dslice = doff[g][:, si * SB:si * SB + nck]
                        din = dslice.unsqueeze(2).broadcast_to([128, nck, 128])
                        iin = iota_bf[:].unsqueeze(1).broadcast_to([128, nck, 128])
                        nc.vector.tensor_tensor(st[:, :nck, :], din, iin,
                                                OP.is_equal)
                        stil[g][si] = st
                    return stil[g][si]

                if SKIP_GATHER:
                    nc.vector.memset(aggr[:], 0.0)
                else:
                    for w in range(NWIN):
                        ps = psA.tile([HID, WIN], f32, name="aggwin",
                                      tag="aggwin")
                        tot = int(K[0][w] + K[1][w])
                        done = 0
                        for g in range(2):
                            for k in range(int(K[g][w])):
                                ck = int(base_ck[g][w]) + k
                                mt = get_msg(g, ck // (GC // 128))
                                st = get_s(g, ck // SB)
                                nc.tensor.matmul(
                                    ps[:],
                                    mt[:, ck % (GC // 128), :],
                                    st[:, ck % SB, :],
                                    start=(done == 0), stop=(done == tot - 1))
                                done += 1
                        n0 = w * WIN
                        n1 = min(NLOC, n0 + WIN)
                        nc.scalar.copy(aggr[:, n0:n1], ps[:, :n1 - n0])

                # ------- combine + relu (h_nxt, un-BN'd) + interleaved
                # ------- node-major transposes / G-partials / table shard
                if PHASE >= 3 and l < L - 1:
                    shard = dTpool.tile([NLOC, HID], bf16, name="shard",
                                        tag="shard")
                gps = psS.tile([G, HID], f32, name="gps", tag="gps")
                for wi in range(NCW):
                    n0 = wi * CWIN
                    n1 = min(NLOC, n0 + CWIN)
                    nn = n1 - n0
                    pc = psC.tile([HID, CWIN], f32, name="cmb", tag="cmb")
                    nc.tensor.matmul(pc[:, :nn], wf[:, 0:HID],
                                     h_cur[:, n0:n1], start=True, stop=False)
                    nc.tensor.matmul(pc[:, :nn], wf[:, HID:2 * HID],
                                     aggr[:, n0:n1], start=False, stop=False)
                    nc.tensor.matmul(pc[:, :nn], rgt[:],
                                     bt[:, n0:n1], start=False, stop=True)
                    nc.scalar.activation(h_nxt[:, n0:n1], pc[:, :nn],
                                         AT.Relu, bias=bias_f[:],
                                         scale=1.0)
                    nc.scalar.square(sq_scr[:, :nn], h_nxt[:, n0:n1])
                    nc.vector.tensor_reduce(
                        sq_acc[:, wi:wi + 1], sq_scr[:, :nn],
                        mybir.AxisListType.X, OP.add)
                    nc.vector.tensor_reduce(
                        sm_acc[:, wi:wi + 1], h_nxt[:, n0:n1],
                        mybir.AxisListType.X, OP.add)
                    if PHASE < 3:
                        continue
                    for t in range(wi * (CWIN // 128),
                                   min(NT, (wi + 1) * (CWIN // 128))):
                        t0 = t * 128
                        t1 = min(NLOC, t0 + 128)
                        tn = t1 - t0
                        tp = psT.tile([128, 128], f32, name="trp", tag="trp")
                        nc.tensor.transpose(tp[:tn, :], h_nxt[:, t0:t1],
                                            ident[:])
                        tr = tpool.tile([128, HID], bf16, name="trs",
                                        tag="trt")
                        nc.scalar.copy(tr[:tn, :], tp[:tn, :])
                        nc.tensor.matmul(gps[:], bnm[:, t * G:(t + 1) * G],
                                         tr[:], start=(t == 0),
                                         stop=(t == NT - 1))
                        if l < L - 1:
                            nc.sync.dma_start(shard[t0:t1, :], tr[:tn, :])
                if PHASE >= 3:
                    nc.scalar.copy(gsb[:], gps[:])
                    if PHASE >= 4 and l < L - 1:
                        rg_start(l + 1)
                    if PHASE >= 4 and l < L - 1:
                        tabn = dTpool.tile([N, HID], bf16, name="tabn",
                                           tag="tabn", addr_space="Shared")
                        nc.gpsimd.collective_compute(
                            "AllGather", mybir.AluOpType.bypass,
                            replica_groups=rg_all,
                            ins=[shard.opt()], outs=[tabn.opt()])
                        tabs.append(tabn)

                if PHASE < 2:
                    h_cur, h_nxt = h_nxt, h_cur
                    continue
                # ---------------- BN stats -> scale/shift (pending affine)
                nc.vector.tensor_reduce(stats[:, 0:1], sm_acc[:],
                                        mybir.AxisListType.X, OP.add)
                nc.vector.tensor_reduce(stats[:, 1:2], sq_acc[:],
                                        mybir.AxisListType.X, OP.add)
                nc.sync.dma_start(st_ins[l][:], stats[:])
                nc.gpsimd.collective_compute(
                    "AllReduce", mybir.AluOpType.add, replica_groups=rg_all,
                    ins=[st_ins[l].opt()], outs=[st_outs[l].opt()])
                nc.sync.dma_start(statsr[:], st_outs[l][:])
                # mean, E[x2], var, std, rstd, scale, shift
                nc.vector.tensor_scalar(sfac[:, 0:2], statsr[:], 1.0 / N, None,
                                        OP.mult)
                nc.vector.tensor_tensor(
                    sfac[:, 2:3], sfac[:, 0:1], sfac[:, 0:1], OP.mult)
                nc.vector.tensor_scalar(sfac[:, 3:4], sfac[:, 1:2],
                                        sfac[:, 2:3], BN_EPS, OP.subtract,
                                        OP.add)
                nc.scalar.sqrt(sfac[:, 4:5], sfac[:, 3:4])
                nc.vector.reciprocal(sfac[:, 5:6], sfac[:, 4:5])
                nc.vector.tensor_scalar(sfac[:, 6:7], aux[:, 3 + l:4 + l],
                                        sfac[:, 5:6], None, OP.mult)
                # shift = beta - mean*scale
                nc.vector.tensor_scalar(sfac[:, 7:8], sfac[:, 0:1],
                                        sfac[:, 6:7], None, OP.mult)
                nc.vector.tensor_sub(sfac[:, 7:8], aux[:, 6 + l:7 + l],
                                     sfac[:, 7:8])

                if PHASE >= 4 and l < L - 1:
                    rg_finish(l + 1)
                if l < L - 1:
                    wf, bias_f = fold_weights(l + 1)
                h_cur, h_nxt = h_nxt, h_cur

            # ---------------- final linear (BN affine folded in)
            lf = fpool.tile([HID, OUT_DIM], f32, name="lf", tag="biasf")
            nc.vector.tensor_scalar(lf[:], lint[:], sfac[:, 6:7], None,
                                    OP.mult)
            plb = psT.tile([OUT_DIM, 1], f32, name="plb", tag="trp")
            nc.tensor.matmul(plb[:], lint[:], sfac[:, 7:8], start=True,
                             stop=True)
            lbf = fpool.tile([OUT_DIM, 1], f32, name="lbf", tag="pac2")
            nc.vector.tensor_add(lbf[:], linb[:], plb[:])
            for wi in range(NCW):
                n0 = wi * CWIN
                n1 = min(NLOC, n0 + CWIN)
                nn = n1 - n0
                po = psC.tile([OUT_DIM, CWIN], f32, name="outp", tag="cmb")
                nc.tensor.matmul(po[:, :nn], lf[:], h_cur[:, n0:n1],
                                 start=True, stop=True)
                ow = opool.tile([OUT_DIM, CWIN], f32, name="ow", tag="ow")
                nc.scalar.activation(ow[:, :nn], po[:, :nn], AT.Identity,
                                     bias=lbf[:], scale=1.0)
                nc.sync.dma_start(out_d[:, n0:n1], ow[:, :nn])

    nc.compile()
    return nc


# ------------------------------------------------------------------- driver
def kernel(**inputs):
    from concourse.bass_utils import run_bass_kernel_spmd

    in_maps, meta = _prep(**inputs)
    nc = _build(meta)
    res = run_bass_kernel_spmd(nc, in_maps, core_ids=list(range(NCORES)))
    out = np.zeros((N, OUT_DIM), dtype=np.float32)
    for c in range(NCORES):
        out[c * NLOC:(c + 1) * NLOC, :] = res.results[c]["out"].T
    return out
